# revision 20
# baseline (speedup 1.0000x reference)
"""Trainium2 Bass kernel for nn_AFBlock (dense transformer block).

Strategy: pure data-parallel over batch (N=8 -> 8 NeuronCores, one batch
element per core, no collectives). All activations are kept feature-major
("transposed", features on SBUF partitions) so every projection contracts
over the partition axis. bf16 matmuls with f32 PSUM accumulation.

Per-core pipeline:
  qkvT = WqkvT.T @ xT (q,k transposed-layout; v normal-layout)
  per head: scoresT = kT_h.T @ qT_h; e = exp(s/sqrt(D)) [ACT, bf16]
            causal mask: above-diag blocks fused into exp bias (-30),
            below-diag as per-partition attention_mask scalar, diagonal
            blocks as a full multiplicative {1, e^-30} mask [DVE]
            Z = ones.T @ e [PE]; rz ~ 1/Z [DVE approx]; rz broadcast via
            bf16 ones matmul; ctxT_h = (v_h.T @ e) * rz ; aT += e * rz
  z0T = WoutT.T @ ctxT + bout + xT ; LN1 (stats via 1/1024-ones matmuls)
  hT = relu(W1T.T @ z1T + b1)  (bounced through DRAM)
  ffT = W2T.T @ hT + b2 + z1T ; LN2 -> zT
Outputs zT / aT are written feature-major; the host transposes on gather.
"""

import os
import numpy as np
import ml_dtypes

import concourse.bass as bass
from concourse import bacc
import concourse.tile as tile
import concourse.mybir as mybir
from concourse.bass_utils import run_bass_kernel_spmd

P = 128
E = 1024
H = 8
D = 128
F = 4096
L = 1024
NB = 8          # batch == number of cores
KE = E // P     # 8 e-chunks
KF = F // P     # 32 f-chunks
LT = 512        # l-tile (psum free dim)
NLT = L // LT   # 2
NDIAG = LT // P  # 4 diagonal-band m'-chunks per l-tile
EPS = 1e-5
SCALE = float(1.0 / np.sqrt(D))
MASKV = float(np.exp(np.float32(-30.0)))

f32 = mybir.dt.float32
bf16 = mybir.dt.bfloat16
bfnp = ml_dtypes.bfloat16
AF = mybir.ActivationFunctionType
ALU = mybir.AluOpType


def build():
    nc = bacc.Bacc(None)

    xTf_d = nc.declare_dram_parameter("xTf", [E, L], f32, False)
    xTb_d = nc.declare_dram_parameter("xTb", [E, L], bf16, False)
    maskd_d = nc.declare_dram_parameter("maskd", [12, P, LT], bf16, False)
    wqk_d = nc.declare_dram_parameter("wqk", [16, P, KE, P], bf16, False)
    wv_d = nc.declare_dram_parameter("wv", [NLT, P, KE, LT], bf16, False)
    bqk_d = nc.declare_dram_parameter("bqk", [P, 16], f32, False)
    bv_d = nc.declare_dram_parameter("bv", [E], f32, False)
    wout_d = nc.declare_dram_parameter("wout", [KE, P, KE, P], bf16, False)
    bout_d = nc.declare_dram_parameter("bout", [P, KE], f32, False)
    w1_d = nc.declare_dram_parameter("w1t", [KF, P, KE, P], bf16, False)
    b1_d = nc.declare_dram_parameter("b1t", [P, KF], f32, False)
    w2_d = nc.declare_dram_parameter("w2t", [KF, P, KE, P], bf16, False)
    b2_d = nc.declare_dram_parameter("b2t", [P, KE], f32, False)
    ln_d = nc.declare_dram_parameter("lnp", [P, KE, 4], f32, False)
    z_d = nc.declare_dram_parameter("z_out", [E, L], f32, isOutput=True)
    a_d = nc.declare_dram_parameter("a_out", [L, L], f32, isOutput=True)

    xTf_r = xTf_d[:].rearrange("(ko ki) l -> ki ko l", ki=P)
    xTb_r = xTb_d[:].rearrange("(ko ki) l -> ki ko l", ki=P)
    z_r = z_d[:].rearrange("(ko ki) l -> ki ko l", ki=P)
    a_r = a_d[:].rearrange("(mo mi) l -> mi mo l", mi=P)

    with tile.TileContext(nc) as tc:
        with (
            tc.tile_pool(name="pool", bufs=1) as pool,
            tc.tile_pool(name="psum", bufs=8, space="PSUM") as psum,
            tc.tile_pool(name="dram", bufs=1, space="DRAM") as dram,
        ):
            def big_b(name):
                return pool.tile([P, KE, L], bf16, tag="bigb", bufs=4, name=name)

            def big_f(name):
                return pool.tile([P, KE, L], f32, tag="bigf", bufs=1, name=name)

            def eblk_t(name):
                return pool.tile([P, KE, LT], bf16, tag="eblk", bufs=4, name=name)

            def pw_t(name):
                return pool.tile([P, KE, P], bf16, tag="pw", bufs=3, name=name)

            def pw2_t(name):
                return pool.tile([P, KE, P], bf16, tag="pw2", bufs=4, name=name)

            def pw4_t(name):
                return pool.tile([P, 4, P], bf16, tag="pw4", bufs=6, name=name)

            def pls_t(name):
                return pool.tile([P, LT], f32, tag="pls", bufs=4, name=name)

            def blt_t(name):
                return pool.tile([P, LT], bf16, tag="blt", bufs=3, name=name)

            def hl_t(name):
                return pool.tile([P, LT], bf16, tag="hl", bufs=8, name=name)

            def rzb_t(name):
                return pool.tile([P, LT], bf16, tag="rzb", bufs=3, name=name)

            def ph_t(name):
                return pool.tile([P, L], bf16, tag="ph", bufs=3, name=name)

            def s32_t(name):
                return pool.tile([32, LT], f32, tag="s32", bufs=4, name=name)

            def s32b_t(name):
                return pool.tile([32, LT], bf16, tag="s32b", bufs=4, name=name)

            def ps_t(name):
                return psum.tile([P, LT], f32, tag="ps", bufs=8, name=name)

            # ---------------- constants ----------------
            bqk_sb = pool.tile([P, 16], f32, tag="bqk", name="bqk_sb")
            nc.gpsimd.dma_start(out=bqk_sb, in_=bqk_d[:])
            bout_sb = pool.tile([P, KE], f32, tag="bout", name="bout_sb")
            nc.gpsimd.dma_start(out=bout_sb, in_=bout_d[:])
            b1_sb = pool.tile([P, KF], f32, tag="b1", name="b1_sb")
            nc.gpsimd.dma_start(out=b1_sb, in_=b1_d[:])
            b2_sb = pool.tile([P, KE], f32, tag="b2", name="b2_sb")
            nc.gpsimd.dma_start(out=b2_sb, in_=b2_d[:])
            ln_sb = pool.tile([P, KE, 4], f32, tag="ln", name="ln_sb")
            nc.gpsimd.dma_start(out=ln_sb, in_=ln_d[:])
            bv_bc = pool.tile([P, E], f32, tag="bvbc", name="bv_bc")
            _bv = bv_d[:]
            bv_ap = bass.AP(tensor=_bv.tensor, offset=_bv.offset,
                            ap=[[0, P]] + list(_bv.ap))
            nc.gpsimd.dma_start(out=bv_bc, in_=bv_ap)

            ones_z = pool.tile([P, 32], bf16, tag="onesz", name="ones_z")
            nc.vector.memset(ones_z, 1.0)
            ones_n = pool.tile([P, 32], bf16, tag="onesn", name="ones_n")
            nc.vector.memset(ones_n, 1.0 / 1024.0)
            ones_r = pool.tile([32, P], bf16, tag="onesr", name="ones_r")
            nc.vector.memset(ones_r, 1.0 / 32.0)
            eps_sb = pool.tile([32, 1], f32, tag="eps", name="eps_sb")
            nc.vector.memset(eps_sb, EPS)
            neg30 = pool.tile([P, 1], f32, tag="neg30", name="neg30")
            nc.vector.memset(neg30, -30.0)

            # ---------------- load x ----------------
            xb = big_b("xb")
            nc.sync.dma_start(out=xb, in_=xTb_r)

            # ---------------- QKV: q, k (transposed layout) ----------------
            qT = big_b("qT")
            kT = big_b("kT")
            for oc in range(16):
                wt = pw_t(f"wqk{oc}")
                nc.sync.dma_start(out=wt, in_=wqk_d[oc])
                dst = qT if oc < 8 else kT
                for lt in range(NLT):
                    ls = slice(lt * LT, (lt + 1) * LT)
                    ps = ps_t(f"ps_qk{oc}_{lt}")
                    for kc in range(KE):
                        nc.tensor.matmul(ps, wt[:, kc, :], xb[:, kc, ls],
                                         start=(kc == 0), stop=(kc == KE - 1))
                    nc.scalar.activation(dst[:, oc % 8, ls], ps, AF.Identity,
                                         bias=bqk_sb[:, oc:oc + 1])

            # ---------------- QKV: v (normal layout) ----------------
            vN = big_b("vN")
            for nt in range(NLT):
                ns = slice(nt * LT, (nt + 1) * LT)
                wvt = eblk_t(f"wv{nt}")
                nc.sync.dma_start(out=wvt, in_=wv_d[nt])
                for lc in range(KE):
                    ps = ps_t(f"ps_v{nt}_{lc}")
                    for kc in range(KE):
                        nc.tensor.matmul(ps, xb[:, kc, lc * P:(lc + 1) * P],
                                         wvt[:, kc, :],
                                         start=(kc == 0), stop=(kc == KE - 1))
                    nc.vector.tensor_add(vN[:, lc, ns], ps, bv_bc[:, ns])

            # ---------------- attention ----------------
            # block type per (lt, mc): diag blocks mc in [4lt, 4lt+3] use the
            # full mask; mc > 4lt+3 is fully above the causal diagonal (all
            # e^-30, fused into the exp bias); mc < 4lt is fully below
            # (mask == attention_mask per partition).
            maskd_sb = pool.tile([P, 12, LT], bf16, tag="maskd",
                                 name="maskd_sb")
            for _i in range(12):
                nc.sync.dma_start(out=maskd_sb[:, _i, :], in_=maskd_d[_i])
            ctxT = big_b("ctxT")
            adram = dram.tile([KE, P, L], bf16, name="adram")

            def attn_scores(lt, h):
                ls = slice(lt * LT, (lt + 1) * LT)
                eb = eblk_t(f"e{lt}_{h}")
                for mc in range(KE):
                    ps = ps_t(f"ps_s{lt}{h}{mc}")
                    nc.tensor.matmul(ps, kT[:, h, mc * P:(mc + 1) * P],
                                     qT[:, h, ls], start=True, stop=True)
                    if mc > NDIAG * lt + (NDIAG - 1):       # above diagonal
                        nc.scalar.activation(eb[:, mc, :], ps, AF.Exp,
                                             scale=SCALE, bias=neg30)
                    else:
                        nc.scalar.activation(eb[:, mc, :], ps, AF.Exp,
                                             scale=SCALE)
                        nc.vector.tensor_mul(
                            eb[:, mc, :], eb[:, mc, :],
                            maskd_sb[:, 4 * lt + mc, :])
                return eb

            def attn_post(lt, h, eb):
                ls = slice(lt * LT, (lt + 1) * LT)
                # softmax denominators (column sums of eb)
                psz = ps_t(f"ps_z{lt}{h}")
                for mc in range(KE):
                    nc.tensor.matmul(psz[0:32, :], ones_z, eb[:, mc, :],
                                     start=(mc == 0), stop=(mc == KE - 1))
                rz32 = s32_t(f"rz32_{lt}{h}")
                nc.vector.reciprocal_approx_fast(rz32, psz[0:32, :])
                rz32b = s32b_t(f"rz32b_{lt}{h}")
                nc.vector.tensor_copy(rz32b, rz32)
                psb = ps_t(f"ps_rb{lt}{h}")
                nc.tensor.matmul(psb, ones_r, rz32b, start=True, stop=True)
                rzbc = rzb_t(f"rzbc{lt}{h}")
                nc.scalar.activation(rzbc, psb, AF.Copy)
                # PV on unnormalized weights (independent of the rz chain)
                psc = ps_t(f"ps_c{lt}{h}")
                for mc in range(KE):
                    nc.tensor.matmul(psc, vN[:, mc, h * P:(h + 1) * P],
                                     eb[:, mc, :],
                                     start=(mc == 0), stop=(mc == KE - 1))
                nc.vector.tensor_mul(ctxT[:, h, ls], psc, rzbc)
                # normalize e in place (after PV consumed it); accumulate the
                # attention-weight mean in DRAM via SWDGE read-modify-write
                for mc in range(KE):
                    nc.vector.tensor_mul(eb[:, mc, :], eb[:, mc, :], rzbc)
                    nc.gpsimd.dma_start(
                        out=adram[mc][:, ls], in_=eb[:, mc, :],
                        accum_op=(ALU.bypass if h == 0 else ALU.add))

            # ---------------- layernorm (256-col segment) ----------------
            LSEG = 256
            NSEG = L // LSEG

            def layernorm_seg(src, out_writer, uid, seg):
                """src: [P, KE, L] f32. Column-wise LN over the feature axis
                for one 256-column segment; out_writer(kc, seg, t_ap)."""
                ls = slice(seg * LSEG, (seg + 1) * LSEG)
                s_b = eblk_t(f"snb{uid}{seg}")
                sq_b = eblk_t(f"sqb{uid}{seg}")
                for kc in range(KE):
                    nc.scalar.activation(s_b[:, kc, :LSEG], src[:, kc, ls],
                                         AF.Copy)
                    nc.vector.tensor_mul(sq_b[:, kc, :LSEG], s_b[:, kc, :LSEG],
                                         s_b[:, kc, :LSEG])
                psmu = ps_t(f"ps_mu{uid}{seg}")
                for kc in range(KE):
                    nc.tensor.matmul(psmu[0:32, :LSEG], ones_n,
                                     s_b[:, kc, :LSEG],
                                     start=(kc == 0), stop=(kc == KE - 1))
                psms = ps_t(f"ps_ms{uid}{seg}")
                for kc in range(KE):
                    nc.tensor.matmul(psms[0:32, :LSEG], ones_n,
                                     sq_b[:, kc, :LSEG],
                                     start=(kc == 0), stop=(kc == KE - 1))
                mu32 = s32_t(f"mu32_{uid}{seg}")
                nc.scalar.activation(mu32[:, :LSEG], psmu[0:32, :LSEG], AF.Copy)
                msq = s32_t(f"msq{uid}{seg}")
                nc.vector.tensor_mul(msq[:, :LSEG], mu32[:, :LSEG],
                                     mu32[:, :LSEG])
                var32 = s32_t(f"var32_{uid}{seg}")
                nc.vector.tensor_tensor(var32[:, :LSEG], psms[0:32, :LSEG],
                                        msq[:, :LSEG], ALU.subtract)
                std32 = s32_t(f"std32_{uid}{seg}")
                nc.scalar.activation(std32[:, :LSEG], var32[:, :LSEG], AF.Sqrt,
                                     bias=eps_sb)
                rstd32 = s32_t(f"rstd32_{uid}{seg}")
                nc.vector.reciprocal_approx_fast(rstd32[:, :LSEG],
                                                 std32[:, :LSEG])
                mu32b = s32b_t(f"mu32b_{uid}{seg}")
                nc.vector.tensor_copy(mu32b[:, :LSEG], mu32[:, :LSEG])
                rstd32b = s32b_t(f"rstd32b_{uid}{seg}")
                nc.vector.tensor_copy(rstd32b[:, :LSEG], rstd32[:, :LSEG])
                psb1 = ps_t(f"ps_b1{uid}{seg}")
                nc.tensor.matmul(psb1[:, :LSEG], ones_r, mu32b[:, :LSEG],
                                 start=True, stop=True)
                mu_bc = pls_t(f"mubc{uid}{seg}")
                nc.scalar.activation(mu_bc[:, :LSEG], psb1[:, :LSEG], AF.Copy)
                psb2 = ps_t(f"ps_b2{uid}{seg}")
                nc.tensor.matmul(psb2[:, :LSEG], ones_r, rstd32b[:, :LSEG],
                                 start=True, stop=True)
                rstd_bc = pls_t(f"rstdbc{uid}{seg}")
                nc.scalar.activation(rstd_bc[:, :LSEG], psb2[:, :LSEG], AF.Copy)
                for kc in range(KE):
                    t = pls_t(f"lnt{uid}{seg}{kc}")
                    nc.vector.tensor_tensor(t[:, :LSEG], src[:, kc, ls],
                                            mu_bc[:, :LSEG], ALU.subtract)
                    nc.vector.tensor_mul(t[:, :LSEG], t[:, :LSEG],
                                         rstd_bc[:, :LSEG])
                    out_writer(kc, seg, t[:, :LSEG])

            # ---------------- per-l-tile downstream stages ----------------
            sT = big_f("sT")
            z1b = big_b("z1b")
            s2 = big_f("s2")
            hdram = dram.tile([KF, P, L], bf16, name="hdram")

            def ln1_writer(kc, seg, t):
                ls = slice(seg * LSEG, (seg + 1) * LSEG)
                nc.scalar.activation(z1b[:, kc, ls], t, AF.Identity,
                                     bias=ln_sb[:, kc, 1:2],
                                     scale=ln_sb[:, kc, 0:1])

            def ln2_writer(kc, seg, t):
                ls = slice(seg * LSEG, (seg + 1) * LSEG)
                zc = pls_t(f"zc{kc}{seg}")
                nc.scalar.activation(zc[:, :LSEG], t, AF.Identity,
                                     bias=ln_sb[:, kc, 3:4],
                                     scale=ln_sb[:, kc, 2:3])
                nc.sync.dma_start(out=z_r[:, kc, ls], in_=zc[:, :LSEG])

            def aout_lt(lt):
                ls = slice(lt * LT, (lt + 1) * LT)
                for mc in range(KE):
                    ab = hl_t(f"ab{mc}{lt}")
                    nc.sync.dma_start(out=ab, in_=adram[mc][:, ls])
                    af = pls_t(f"af{mc}{lt}")
                    nc.scalar.activation(af, ab, AF.Copy, scale=1.0 / H)
                    nc.sync.dma_start(out=a_r[:, mc, ls], in_=af)

            def outproj_lt(lt):
                ls = slice(lt * LT, (lt + 1) * LT)
                for oc in range(KE):
                    wt = pw_t(f"wo{lt}{oc}")
                    nc.sync.dma_start(out=wt, in_=wout_d[oc])
                    ps = ps_t(f"ps_o{oc}{lt}")
                    for kc in range(KE):
                        nc.tensor.matmul(ps, wt[:, kc, :], ctxT[:, kc, ls],
                                         start=(kc == 0), stop=(kc == KE - 1))
                    nc.scalar.activation(sT[:, oc, ls], ps, AF.Identity,
                                         bias=bout_sb[:, oc:oc + 1])
                    xc = pls_t(f"xc{oc}{lt}")
                    nc.sync.dma_start(out=xc, in_=xTf_r[:, oc, ls])
                    nc.vector.tensor_add(sT[:, oc, ls], sT[:, oc, ls], xc)

            def mlp1_lt(lt):
                ls = slice(lt * LT, (lt + 1) * LT)
                for fc in range(KF):
                    wt = pw_t(f"w1_{lt}_{fc}")
                    nc.sync.dma_start(out=wt, in_=w1_d[fc])
                    ps = ps_t(f"ps_h{fc}{lt}")
                    for kc in range(KE):
                        nc.tensor.matmul(ps, wt[:, kc, :], z1b[:, kc, ls],
                                         start=(kc == 0), stop=(kc == KE - 1))
                    hc = hl_t(f"hc{fc}{lt}")
                    nc.scalar.activation(hc, ps, AF.Relu,
                                         bias=b1_sb[:, fc:fc + 1])
                    nc.sync.dma_start(out=hdram[fc][:, ls], in_=hc)

            def mlp2_lt(lt):
                ls = slice(lt * LT, (lt + 1) * LT)
                pss = [ps_t(f"ps_ff{lt}{ec}") for ec in range(KE)]
                for kc2 in range(KF):
                    hcl = hl_t(f"hcl{lt}{kc2}")
                    nc.sync.dma_start(out=hcl, in_=hdram[kc2][:, ls])
                    w2t = pw2_t(f"w2_{lt}{kc2}")
                    nc.sync.dma_start(out=w2t, in_=w2_d[kc2])
                    for ec in range(KE):
                        nc.tensor.matmul(pss[ec], w2t[:, ec, :], hcl,
                                         start=(kc2 == 0), stop=(kc2 == KF - 1))
                for ec in range(KE):
                    nc.scalar.activation(s2[:, ec, ls], pss[ec], AF.Identity,
                                         bias=b2_sb[:, ec:ec + 1])
                    nc.vector.tensor_add(s2[:, ec, ls], s2[:, ec, ls],
                                         z1b[:, ec, ls])

            # ---------------- global schedule ----------------
            for lt in range(NLT):
                pending = None
                for h in range(H):
                    eb = attn_scores(lt, h)
                    if pending is not None:
                        attn_post(*pending)
                    pending = (lt, h, eb)
                attn_post(*pending)
                aout_lt(lt)
                outproj_lt(lt)
                for seg in (2 * lt, 2 * lt + 1):
                    layernorm_seg(sT, ln1_writer, "a", seg)
                if lt == 1:
                    mlp1_lt(0)
            mlp1_lt(1)
            for lt in range(NLT):
                mlp2_lt(lt)
                for seg in (2 * lt, 2 * lt + 1):
                    layernorm_seg(s2, ln2_writer, "b", seg)

    nc.compile()
    return nc


def _prep_shared(inputs):
    """Host-side weight preparation (shared across cores)."""
    def bfc(a):
        return np.ascontiguousarray(a.astype(bfnp))

    def f32c(a):
        return np.ascontiguousarray(a.astype(np.float32))

    WqkvT = np.asarray(inputs["in_proj_w"], np.float32).T  # (E, 3E)
    wqk = bfc(WqkvT[:, :2 * E].reshape(KE, P, 16, P).transpose(2, 1, 0, 3))
    wv = bfc(WqkvT[:, 2 * E:].reshape(KE, P, NLT, LT).transpose(2, 1, 0, 3))
    ipb = np.asarray(inputs["in_proj_b"], np.float32)
    bqk = f32c(ipb[:2 * E].reshape(16, P).T)
    bv = f32c(ipb[2 * E:])
    WoutT = np.asarray(inputs["out_w"], np.float32).T
    wout = bfc(WoutT.reshape(KE, P, KE, P).transpose(2, 1, 0, 3))
    bout = f32c(np.asarray(inputs["out_b"], np.float32).reshape(KE, P).T)
    W1T = np.asarray(inputs["w1"], np.float32).T  # (E, F)
    w1t = bfc(W1T.reshape(KE, P, KF, P).transpose(2, 1, 0, 3))
    b1t = f32c(np.asarray(inputs["b1"], np.float32).reshape(KF, P).T)
    W2T = np.asarray(inputs["w2"], np.float32).T  # (F, E)
    w2t = bfc(W2T.reshape(KF, P, KE, P))
    b2t = f32c(np.asarray(inputs["b2"], np.float32).reshape(KE, P).T)
    lnp = np.stack([np.asarray(inputs[k], np.float32).reshape(KE, P).T
                    for k in ("ln1_g", "ln1_b", "ln2_g", "ln2_b")], axis=-1)
    lnp = np.ascontiguousarray(lnp)  # [P, KE, 4]
    return dict(wqk=wqk, wv=wv, bqk=bqk, bv=bv, wout=wout, bout=bout,
                w1t=w1t, b1t=b1t, w2t=w2t, b2t=b2t, lnp=lnp)


_TRIL = None


def make_in_map(x_b, am_b, shared):
    """Per-core input dict from one batch element."""
    global _TRIL
    if _TRIL is None:
        _TRIL = np.tril(np.ones((L, L), np.float32))
    xT = np.ascontiguousarray(np.asarray(x_b, np.float32).T)  # (E, L) f32
    keepT = (_TRIL * np.asarray(am_b).astype(np.float32)[None, :]).T  # [m', l]
    maskT = np.where(keepT > 0.5, np.float32(1.0),
                     np.float32(MASKV)).astype(bfnp)
    # diagonal-band blocks only: maskd[lt, :, j, :] is the block for
    # m'-chunk mc = 4*lt + j, columns [lt*LT, (lt+1)*LT)
    maskd = np.empty((12, P, LT), dtype=bfnp)
    for lt in range(NLT):
        for mc in range(NDIAG * (lt + 1)):
            maskd[4 * lt + mc] = maskT[mc * P:(mc + 1) * P,
                                       lt * LT:(lt + 1) * LT]
    m = dict(shared)
    m["xTf"] = xT
    m["xTb"] = np.ascontiguousarray(xT.astype(bfnp))
    m["maskd"] = np.ascontiguousarray(maskd)
    return m


_LAST_RESULT = {}


def kernel(**inputs):
    x = np.asarray(inputs["x"], np.float32)             # (NB, L, E)
    am = np.asarray(inputs["attention_mask"])           # (NB, L)
    shared = _prep_shared(inputs)

    in_maps = [make_in_map(x[b], am[b], shared) for b in range(NB)]

    nc = build()
    trace = os.environ.get("AF_KERNEL_TRACE", "0") == "1"
    res = run_bass_kernel_spmd(nc, in_maps, core_ids=list(range(NB)),
                               trace=trace)
    _LAST_RESULT["exec_time_ns"] = res.exec_time_ns
    _LAST_RESULT["res"] = res

    z = np.stack([np.asarray(res.results[b]["z_out"]).T for b in range(NB)])
    a = np.stack([np.asarray(res.results[b]["a_out"]).T for b in range(NB)])
    return z.astype(np.float32), a.astype(np.float32)


# revision 21
# speedup vs baseline: 1.2540x; 1.2540x over previous
"""Trainium2 Bass kernel for nn_AFBlock (dense transformer block).

Strategy: pure data-parallel over batch (N=8 -> 8 NeuronCores, one batch
element per core, no collectives). All activations are kept feature-major
("transposed", features on SBUF partitions) so every projection contracts
over the partition axis. bf16 matmuls with f32 PSUM accumulation.

Per-core pipeline:
  qkvT = WqkvT.T @ xT (q,k transposed-layout; v normal-layout)
  per head: scoresT = kT_h.T @ qT_h; e = exp(s/sqrt(D)) [ACT, bf16]
            causal mask: above-diag blocks fused into exp bias (-30),
            below-diag as per-partition attention_mask scalar, diagonal
            blocks as a full multiplicative {1, e^-30} mask [DVE]
            Z = ones.T @ e [PE]; rz ~ 1/Z [DVE approx]; rz broadcast via
            bf16 ones matmul; ctxT_h = (v_h.T @ e) * rz ; aT += e * rz
  z0T = WoutT.T @ ctxT + bout + xT ; LN1 (stats via 1/1024-ones matmuls)
  hT = relu(W1T.T @ z1T + b1)  (bounced through DRAM)
  ffT = W2T.T @ hT + b2 + z1T ; LN2 -> zT
Outputs zT / aT are written feature-major; the host transposes on gather.
"""

import os
import numpy as np
import ml_dtypes

import concourse.bass as bass
from concourse import bacc
import concourse.tile as tile
import concourse.mybir as mybir
from concourse.bass_utils import run_bass_kernel_spmd

P = 128
E = 1024
H = 8
D = 128
F = 4096
L = 1024
NB = 8          # batch == number of cores
KE = E // P     # 8 e-chunks
KF = F // P     # 32 f-chunks
LT = 512        # l-tile (psum free dim)
NLT = L // LT   # 2
NDIAG = LT // P  # 4 diagonal-band m'-chunks per l-tile
EPS = 1e-5
SCALE = float(1.0 / np.sqrt(D))
MASKV = float(np.exp(np.float32(-30.0)))

f32 = mybir.dt.float32
bf16 = mybir.dt.bfloat16
bfnp = ml_dtypes.bfloat16
AF = mybir.ActivationFunctionType
ALU = mybir.AluOpType


def build():
    nc = bacc.Bacc(None)

    xTf_d = nc.declare_dram_parameter("xTf", [E, L], f32, False)
    xTb_d = nc.declare_dram_parameter("xTb", [E, L], bf16, False)
    maskd_d = nc.declare_dram_parameter("maskd", [12, P, LT], bf16, False)
    wqk_d = nc.declare_dram_parameter("wqk", [16, P, KE, P], bf16, False)
    wv_d = nc.declare_dram_parameter("wv", [NLT, P, KE, LT], bf16, False)
    bqk_d = nc.declare_dram_parameter("bqk", [P, 16], f32, False)
    bv_d = nc.declare_dram_parameter("bv", [E], f32, False)
    wout_d = nc.declare_dram_parameter("wout", [KE, P, KE, P], bf16, False)
    bout_d = nc.declare_dram_parameter("bout", [P, KE], f32, False)
    w1_d = nc.declare_dram_parameter("w1t", [KF, P, KE, P], bf16, False)
    b1_d = nc.declare_dram_parameter("b1t", [P, KF], f32, False)
    w2_d = nc.declare_dram_parameter("w2t", [KF, P, KE, P], bf16, False)
    b2_d = nc.declare_dram_parameter("b2t", [P, KE], f32, False)
    ln_d = nc.declare_dram_parameter("lnp", [P, KE, 4], f32, False)
    z_d = nc.declare_dram_parameter("z_out", [E, L], f32, isOutput=True)
    a_d = nc.declare_dram_parameter("a_out", [L, L], f32, isOutput=True)

    xTf_r = xTf_d[:].rearrange("(ko ki) l -> ki ko l", ki=P)
    xTb_r = xTb_d[:].rearrange("(ko ki) l -> ki ko l", ki=P)
    z_r = z_d[:].rearrange("(ko ki) l -> ki ko l", ki=P)
    a_r = a_d[:].rearrange("(mo mi) l -> mi mo l", mi=P)

    with tile.TileContext(nc) as tc:
        with (
            tc.tile_pool(name="pool", bufs=1) as pool,
            tc.tile_pool(name="psum", bufs=8, space="PSUM") as psum,
            tc.tile_pool(name="dram", bufs=1, space="DRAM") as dram,
        ):
            def big_b(name):
                return pool.tile([P, KE, L], bf16, tag="bigb", bufs=4, name=name)

            def big_f(name):
                return pool.tile([P, KE, L], f32, tag="bigf", bufs=1, name=name)

            def eblk_t(name):
                return pool.tile([P, KE, LT], bf16, tag="eblk", bufs=4, name=name)

            def pw_t(name):
                return pool.tile([P, KE, P], bf16, tag="pw", bufs=3, name=name)

            def pw2_t(name):
                return pool.tile([P, KE, P], bf16, tag="pw2", bufs=4, name=name)

            def pw4_t(name):
                return pool.tile([P, 4, P], bf16, tag="pw4", bufs=6, name=name)

            def pls_t(name):
                return pool.tile([P, LT], f32, tag="pls", bufs=4, name=name)

            def blt_t(name):
                return pool.tile([P, LT], bf16, tag="blt", bufs=3, name=name)

            def hl_t(name):
                return pool.tile([P, LT], bf16, tag="hl", bufs=8, name=name)

            def rzb_t(name):
                return pool.tile([P, LT], bf16, tag="rzb", bufs=3, name=name)

            def ph_t(name):
                return pool.tile([P, L], bf16, tag="ph", bufs=3, name=name)

            def s32_t(name):
                return pool.tile([32, LT], f32, tag="s32", bufs=4, name=name)

            def s32b_t(name):
                return pool.tile([32, LT], bf16, tag="s32b", bufs=4, name=name)

            def ps_t(name):
                return psum.tile([P, LT], f32, tag="ps", bufs=8, name=name)

            # ---------------- constants ----------------
            bqk_sb = pool.tile([P, 16], f32, tag="bqk", name="bqk_sb")
            nc.gpsimd.dma_start(out=bqk_sb, in_=bqk_d[:])
            bout_sb = pool.tile([P, KE], f32, tag="bout", name="bout_sb")
            nc.gpsimd.dma_start(out=bout_sb, in_=bout_d[:])
            b1_sb = pool.tile([P, KF], f32, tag="b1", name="b1_sb")
            nc.gpsimd.dma_start(out=b1_sb, in_=b1_d[:])
            b2_sb = pool.tile([P, KE], f32, tag="b2", name="b2_sb")
            nc.gpsimd.dma_start(out=b2_sb, in_=b2_d[:])
            ln_sb = pool.tile([P, KE, 4], f32, tag="ln", name="ln_sb")
            nc.gpsimd.dma_start(out=ln_sb, in_=ln_d[:])
            bv_bc = pool.tile([P, E], f32, tag="bvbc", name="bv_bc")
            _bv = bv_d[:]
            bv_ap = bass.AP(tensor=_bv.tensor, offset=_bv.offset,
                            ap=[[0, P]] + list(_bv.ap))
            nc.gpsimd.dma_start(out=bv_bc, in_=bv_ap)

            ones_z = pool.tile([P, 32], bf16, tag="onesz", name="ones_z")
            nc.vector.memset(ones_z, 1.0)
            ones_n = pool.tile([P, 32], bf16, tag="onesn", name="ones_n")
            nc.vector.memset(ones_n, 1.0 / 1024.0)
            ones_r = pool.tile([32, P], bf16, tag="onesr", name="ones_r")
            nc.vector.memset(ones_r, 1.0 / 32.0)
            eps_sb = pool.tile([32, 1], f32, tag="eps", name="eps_sb")
            nc.vector.memset(eps_sb, EPS)
            neg30 = pool.tile([P, 1], f32, tag="neg30", name="neg30")
            nc.vector.memset(neg30, -30.0)

            # ---------------- load x ----------------
            xb = big_b("xb")
            nc.sync.dma_start(out=xb, in_=xTb_r)

            # ---------------- QKV: q, k (transposed layout) ----------------
            qT = big_b("qT")
            kT = big_b("kT")
            for oc in range(16):
                wt = pw_t(f"wqk{oc}")
                nc.sync.dma_start(out=wt, in_=wqk_d[oc])
                dst = qT if oc < 8 else kT
                for lt in range(NLT):
                    ls = slice(lt * LT, (lt + 1) * LT)
                    ps = ps_t(f"ps_qk{oc}_{lt}")
                    for kc in range(KE):
                        nc.tensor.matmul(ps, wt[:, kc, :], xb[:, kc, ls],
                                         start=(kc == 0), stop=(kc == KE - 1))
                    nc.scalar.activation(dst[:, oc % 8, ls], ps, AF.Identity,
                                         bias=bqk_sb[:, oc:oc + 1])

            # ---------------- QKV: v (normal layout) ----------------
            vN = big_b("vN")
            for nt in range(NLT):
                ns = slice(nt * LT, (nt + 1) * LT)
                wvt = eblk_t(f"wv{nt}")
                nc.sync.dma_start(out=wvt, in_=wv_d[nt])
                for lc in range(KE):
                    ps = ps_t(f"ps_v{nt}_{lc}")
                    for kc in range(KE):
                        nc.tensor.matmul(ps, xb[:, kc, lc * P:(lc + 1) * P],
                                         wvt[:, kc, :],
                                         start=(kc == 0), stop=(kc == KE - 1))
                    nc.vector.tensor_add(vN[:, lc, ns], ps, bv_bc[:, ns])

            # ---------------- attention ----------------
            # block type per (lt, mc): diag blocks mc in [4lt, 4lt+3] use the
            # full mask; mc > 4lt+3 is fully above the causal diagonal (all
            # e^-30, fused into the exp bias); mc < 4lt is fully below
            # (mask == attention_mask per partition).
            maskd_sb = pool.tile([P, 12, LT], bf16, tag="maskd",
                                 name="maskd_sb")
            for _i in range(12):
                nc.sync.dma_start(out=maskd_sb[:, _i, :], in_=maskd_d[_i])
            ctxT = big_b("ctxT")
            adram = dram.tile([KE, P, L], bf16, name="adram")

            def attn_scores(lt, h):
                ls = slice(lt * LT, (lt + 1) * LT)
                eb = eblk_t(f"e{lt}_{h}")
                for mc in range(KE):
                    ps = ps_t(f"ps_s{lt}{h}{mc}")
                    nc.tensor.matmul(ps, kT[:, h, mc * P:(mc + 1) * P],
                                     qT[:, h, ls], start=True, stop=True)
                    if mc > NDIAG * lt + (NDIAG - 1):       # above diagonal
                        nc.scalar.activation(eb[:, mc, :], ps, AF.Exp,
                                             scale=SCALE, bias=neg30)
                    else:
                        nc.scalar.activation(eb[:, mc, :], ps, AF.Exp,
                                             scale=SCALE)
                        nc.vector.tensor_mul(
                            eb[:, mc, :], eb[:, mc, :],
                            maskd_sb[:, 4 * lt + mc, :])
                return eb

            def attn_post(lt, h, eb):
                ls = slice(lt * LT, (lt + 1) * LT)
                # softmax denominators (column sums of eb)
                psz = ps_t(f"ps_z{lt}{h}")
                for mc in range(KE):
                    nc.tensor.matmul(psz[0:32, :], ones_z, eb[:, mc, :],
                                     start=(mc == 0), stop=(mc == KE - 1))
                rz32 = s32_t(f"rz32_{lt}{h}")
                nc.vector.reciprocal_approx_fast(rz32, psz[0:32, :])
                rz32b = s32b_t(f"rz32b_{lt}{h}")
                nc.vector.tensor_copy(rz32b, rz32)
                psb = ps_t(f"ps_rb{lt}{h}")
                nc.tensor.matmul(psb, ones_r, rz32b, start=True, stop=True)
                rzbc = rzb_t(f"rzbc{lt}{h}")
                nc.scalar.activation(rzbc, psb, AF.Copy)
                # PV on unnormalized weights (independent of the rz chain)
                psc = ps_t(f"ps_c{lt}{h}")
                for mc in range(KE):
                    nc.tensor.matmul(psc, vN[:, mc, h * P:(h + 1) * P],
                                     eb[:, mc, :],
                                     start=(mc == 0), stop=(mc == KE - 1))
                nc.vector.tensor_mul(ctxT[:, h, ls], psc, rzbc)
                # normalize e in place (after PV consumed it); accumulate the
                # attention-weight mean in DRAM via SWDGE read-modify-write
                for mc in range(KE):
                    nc.vector.tensor_mul(eb[:, mc, :], eb[:, mc, :], rzbc)
                    nc.gpsimd.dma_start(
                        out=adram[mc][:, ls], in_=eb[:, mc, :],
                        accum_op=(ALU.bypass if h == 0 else ALU.add))

            # ---------------- layernorm (256-col segment) ----------------
            LSEG = 256
            NSEG = L // LSEG

            def layernorm_seg(src, out_writer, uid, seg):
                """src: [P, KE, L] f32. Column-wise LN over the feature axis
                for one 256-column segment; out_writer(kc, seg, t_ap)."""
                ls = slice(seg * LSEG, (seg + 1) * LSEG)
                s_b = eblk_t(f"snb{uid}{seg}")
                sq_b = eblk_t(f"sqb{uid}{seg}")
                for kc in range(KE):
                    nc.scalar.activation(s_b[:, kc, :LSEG], src[:, kc, ls],
                                         AF.Copy)
                    nc.vector.tensor_mul(sq_b[:, kc, :LSEG], s_b[:, kc, :LSEG],
                                         s_b[:, kc, :LSEG])
                psmu = ps_t(f"ps_mu{uid}{seg}")
                for kc in range(KE):
                    nc.tensor.matmul(psmu[0:32, :LSEG], ones_n,
                                     s_b[:, kc, :LSEG],
                                     start=(kc == 0), stop=(kc == KE - 1))
                psms = ps_t(f"ps_ms{uid}{seg}")
                for kc in range(KE):
                    nc.tensor.matmul(psms[0:32, :LSEG], ones_n,
                                     sq_b[:, kc, :LSEG],
                                     start=(kc == 0), stop=(kc == KE - 1))
                mu32 = s32_t(f"mu32_{uid}{seg}")
                nc.scalar.activation(mu32[:, :LSEG], psmu[0:32, :LSEG], AF.Copy)
                msq = s32_t(f"msq{uid}{seg}")
                nc.vector.tensor_mul(msq[:, :LSEG], mu32[:, :LSEG],
                                     mu32[:, :LSEG])
                var32 = s32_t(f"var32_{uid}{seg}")
                nc.vector.tensor_tensor(var32[:, :LSEG], psms[0:32, :LSEG],
                                        msq[:, :LSEG], ALU.subtract)
                std32 = s32_t(f"std32_{uid}{seg}")
                nc.scalar.activation(std32[:, :LSEG], var32[:, :LSEG], AF.Sqrt,
                                     bias=eps_sb)
                rstd32 = s32_t(f"rstd32_{uid}{seg}")
                nc.vector.reciprocal_approx_fast(rstd32[:, :LSEG],
                                                 std32[:, :LSEG])
                mu32b = s32b_t(f"mu32b_{uid}{seg}")
                nc.vector.tensor_copy(mu32b[:, :LSEG], mu32[:, :LSEG])
                rstd32b = s32b_t(f"rstd32b_{uid}{seg}")
                nc.vector.tensor_copy(rstd32b[:, :LSEG], rstd32[:, :LSEG])
                psb1 = ps_t(f"ps_b1{uid}{seg}")
                nc.tensor.matmul(psb1[:, :LSEG], ones_r, mu32b[:, :LSEG],
                                 start=True, stop=True)
                mu_bc = pls_t(f"mubc{uid}{seg}")
                nc.scalar.activation(mu_bc[:, :LSEG], psb1[:, :LSEG], AF.Copy)
                psb2 = ps_t(f"ps_b2{uid}{seg}")
                nc.tensor.matmul(psb2[:, :LSEG], ones_r, rstd32b[:, :LSEG],
                                 start=True, stop=True)
                rstd_bc = pls_t(f"rstdbc{uid}{seg}")
                nc.scalar.activation(rstd_bc[:, :LSEG], psb2[:, :LSEG], AF.Copy)
                for kc in range(KE):
                    t = pls_t(f"lnt{uid}{seg}{kc}")
                    nc.vector.tensor_tensor(t[:, :LSEG], src[:, kc, ls],
                                            mu_bc[:, :LSEG], ALU.subtract)
                    nc.vector.tensor_mul(t[:, :LSEG], t[:, :LSEG],
                                         rstd_bc[:, :LSEG])
                    out_writer(kc, seg, t[:, :LSEG])

            # ---------------- per-l-tile downstream stages ----------------
            sT = big_f("sT")
            z1b = big_b("z1b")
            s2 = big_f("s2")
            hdram = dram.tile([KF, P, L], bf16, name="hdram")

            def ln1_writer(kc, seg, t):
                ls = slice(seg * LSEG, (seg + 1) * LSEG)
                nc.scalar.activation(z1b[:, kc, ls], t, AF.Identity,
                                     bias=ln_sb[:, kc, 1:2],
                                     scale=ln_sb[:, kc, 0:1])

            def ln2_writer(kc, seg, t):
                ls = slice(seg * LSEG, (seg + 1) * LSEG)
                zc = pls_t(f"zc{kc}{seg}")
                nc.scalar.activation(zc[:, :LSEG], t, AF.Identity,
                                     bias=ln_sb[:, kc, 3:4],
                                     scale=ln_sb[:, kc, 2:3])
                nc.sync.dma_start(out=z_r[:, kc, ls], in_=zc[:, :LSEG])

            def aout_lt(lt):
                ls = slice(lt * LT, (lt + 1) * LT)
                for mc in range(KE):
                    ab = hl_t(f"ab{mc}{lt}")
                    nc.sync.dma_start(out=ab, in_=adram[mc][:, ls])
                    af = pls_t(f"af{mc}{lt}")
                    nc.scalar.activation(af, ab, AF.Copy, scale=1.0 / H)
                    nc.sync.dma_start(out=a_r[:, mc, ls], in_=af)

            def outproj_lt(lt):
                ls = slice(lt * LT, (lt + 1) * LT)
                for oc in range(KE):
                    wt = pw_t(f"wo{lt}{oc}")
                    nc.sync.dma_start(out=wt, in_=wout_d[oc])
                    ps = ps_t(f"ps_o{oc}{lt}")
                    for kc in range(KE):
                        nc.tensor.matmul(ps, wt[:, kc, :], ctxT[:, kc, ls],
                                         start=(kc == 0), stop=(kc == KE - 1))
                    nc.scalar.activation(sT[:, oc, ls], ps, AF.Identity,
                                         bias=bout_sb[:, oc:oc + 1])
                    xc = pls_t(f"xc{oc}{lt}")
                    nc.sync.dma_start(out=xc, in_=xTf_r[:, oc, ls])
                    nc.vector.tensor_add(sT[:, oc, ls], sT[:, oc, ls], xc)

            def mlp1_lt(lt):
                ls = slice(lt * LT, (lt + 1) * LT)
                for fc in range(KF):
                    wt = pw_t(f"w1_{lt}_{fc}")
                    nc.sync.dma_start(out=wt, in_=w1_d[fc])
                    ps = ps_t(f"ps_h{fc}{lt}")
                    for kc in range(KE):
                        nc.tensor.matmul(ps, wt[:, kc, :], z1b[:, kc, ls],
                                         start=(kc == 0), stop=(kc == KE - 1))
                    hc = hl_t(f"hc{fc}{lt}")
                    nc.scalar.activation(hc, ps, AF.Relu,
                                         bias=b1_sb[:, fc:fc + 1])
                    nc.sync.dma_start(out=hdram[fc][:, ls], in_=hc)

            def mlp2_lt(lt):
                ls = slice(lt * LT, (lt + 1) * LT)
                pss = [ps_t(f"ps_ff{lt}{ec}") for ec in range(KE)]
                for kc2 in range(KF):
                    hcl = hl_t(f"hcl{lt}{kc2}")
                    nc.sync.dma_start(out=hcl, in_=hdram[kc2][:, ls])
                    w2t = pw2_t(f"w2_{lt}{kc2}")
                    nc.sync.dma_start(out=w2t, in_=w2_d[kc2])
                    for ec in range(KE):
                        nc.tensor.matmul(pss[ec], w2t[:, ec, :], hcl,
                                         start=(kc2 == 0), stop=(kc2 == KF - 1))
                for ec in range(KE):
                    nc.scalar.activation(s2[:, ec, ls], pss[ec], AF.Identity,
                                         bias=b2_sb[:, ec:ec + 1])
                    nc.vector.tensor_add(s2[:, ec, ls], s2[:, ec, ls],
                                         z1b[:, ec, ls])

            # ---------------- global schedule ----------------
            pending = None
            for lt in range(NLT):
                for h in range(H):
                    eb = attn_scores(lt, h)
                    if pending is not None:
                        attn_post(*pending)
                    pending = (lt, h, eb)
            attn_post(*pending)
            for lt in range(NLT):
                aout_lt(lt)
            for oc in range(KE):
                wt = pw_t(f"wo{oc}")
                nc.sync.dma_start(out=wt, in_=wout_d[oc])
                for lt in range(NLT):
                    ls = slice(lt * LT, (lt + 1) * LT)
                    ps = ps_t(f"ps_o{oc}{lt}")
                    for kc in range(KE):
                        nc.tensor.matmul(ps, wt[:, kc, :], ctxT[:, kc, ls],
                                         start=(kc == 0), stop=(kc == KE - 1))
                    nc.scalar.activation(sT[:, oc, ls], ps, AF.Identity,
                                         bias=bout_sb[:, oc:oc + 1])
                    xc = pls_t(f"xc{oc}{lt}")
                    nc.sync.dma_start(out=xc, in_=xTf_r[:, oc, ls])
                    nc.vector.tensor_add(sT[:, oc, ls], sT[:, oc, ls], xc)
            for seg in range(NSEG):
                layernorm_seg(sT, ln1_writer, "a", seg)
            for fc in range(KF):
                wt = pw_t(f"w1_{fc}")
                nc.sync.dma_start(out=wt, in_=w1_d[fc])
                hc = ph_t(f"hc{fc}")
                for lt in range(NLT):
                    ls = slice(lt * LT, (lt + 1) * LT)
                    ps = ps_t(f"ps_h{fc}{lt}")
                    for kc in range(KE):
                        nc.tensor.matmul(ps, wt[:, kc, :], z1b[:, kc, ls],
                                         start=(kc == 0), stop=(kc == KE - 1))
                    nc.scalar.activation(hc[:, ls], ps, AF.Relu,
                                         bias=b1_sb[:, fc:fc + 1])
                nc.sync.dma_start(out=hdram[fc], in_=hc)
            for lt in range(NLT):
                mlp2_lt(lt)
                for seg in (2 * lt, 2 * lt + 1):
                    layernorm_seg(s2, ln2_writer, "b", seg)

    nc.compile()
    return nc


def _prep_shared(inputs):
    """Host-side weight preparation (shared across cores)."""
    def bfc(a):
        return np.ascontiguousarray(a.astype(bfnp))

    def f32c(a):
        return np.ascontiguousarray(a.astype(np.float32))

    WqkvT = np.asarray(inputs["in_proj_w"], np.float32).T  # (E, 3E)
    wqk = bfc(WqkvT[:, :2 * E].reshape(KE, P, 16, P).transpose(2, 1, 0, 3))
    wv = bfc(WqkvT[:, 2 * E:].reshape(KE, P, NLT, LT).transpose(2, 1, 0, 3))
    ipb = np.asarray(inputs["in_proj_b"], np.float32)
    bqk = f32c(ipb[:2 * E].reshape(16, P).T)
    bv = f32c(ipb[2 * E:])
    WoutT = np.asarray(inputs["out_w"], np.float32).T
    wout = bfc(WoutT.reshape(KE, P, KE, P).transpose(2, 1, 0, 3))
    bout = f32c(np.asarray(inputs["out_b"], np.float32).reshape(KE, P).T)
    W1T = np.asarray(inputs["w1"], np.float32).T  # (E, F)
    w1t = bfc(W1T.reshape(KE, P, KF, P).transpose(2, 1, 0, 3))
    b1t = f32c(np.asarray(inputs["b1"], np.float32).reshape(KF, P).T)
    W2T = np.asarray(inputs["w2"], np.float32).T  # (F, E)
    w2t = bfc(W2T.reshape(KF, P, KE, P))
    b2t = f32c(np.asarray(inputs["b2"], np.float32).reshape(KE, P).T)
    lnp = np.stack([np.asarray(inputs[k], np.float32).reshape(KE, P).T
                    for k in ("ln1_g", "ln1_b", "ln2_g", "ln2_b")], axis=-1)
    lnp = np.ascontiguousarray(lnp)  # [P, KE, 4]
    return dict(wqk=wqk, wv=wv, bqk=bqk, bv=bv, wout=wout, bout=bout,
                w1t=w1t, b1t=b1t, w2t=w2t, b2t=b2t, lnp=lnp)


_TRIL = None


def make_in_map(x_b, am_b, shared):
    """Per-core input dict from one batch element."""
    global _TRIL
    if _TRIL is None:
        _TRIL = np.tril(np.ones((L, L), np.float32))
    xT = np.ascontiguousarray(np.asarray(x_b, np.float32).T)  # (E, L) f32
    keepT = (_TRIL * np.asarray(am_b).astype(np.float32)[None, :]).T  # [m', l]
    maskT = np.where(keepT > 0.5, np.float32(1.0),
                     np.float32(MASKV)).astype(bfnp)
    # diagonal-band blocks only: maskd[lt, :, j, :] is the block for
    # m'-chunk mc = 4*lt + j, columns [lt*LT, (lt+1)*LT)
    maskd = np.empty((12, P, LT), dtype=bfnp)
    for lt in range(NLT):
        for mc in range(NDIAG * (lt + 1)):
            maskd[4 * lt + mc] = maskT[mc * P:(mc + 1) * P,
                                       lt * LT:(lt + 1) * LT]
    m = dict(shared)
    m["xTf"] = xT
    m["xTb"] = np.ascontiguousarray(xT.astype(bfnp))
    m["maskd"] = np.ascontiguousarray(maskd)
    return m


_LAST_RESULT = {}


def kernel(**inputs):
    x = np.asarray(inputs["x"], np.float32)             # (NB, L, E)
    am = np.asarray(inputs["attention_mask"])           # (NB, L)
    shared = _prep_shared(inputs)

    in_maps = [make_in_map(x[b], am[b], shared) for b in range(NB)]

    nc = build()
    trace = os.environ.get("AF_KERNEL_TRACE", "0") == "1"
    res = run_bass_kernel_spmd(nc, in_maps, core_ids=list(range(NB)),
                               trace=trace)
    _LAST_RESULT["exec_time_ns"] = res.exec_time_ns
    _LAST_RESULT["res"] = res

    z = np.stack([np.asarray(res.results[b]["z_out"]).T for b in range(NB)])
    a = np.stack([np.asarray(res.results[b]["a_out"]).T for b in range(NB)])
    return z.astype(np.float32), a.astype(np.float32)


# revision 22
# speedup vs baseline: 1.3450x; 1.0725x over previous
"""Trainium2 Bass kernel for nn_AFBlock (dense transformer block).

Strategy: pure data-parallel over batch (N=8 -> 8 NeuronCores, one batch
element per core, no collectives). All activations are kept feature-major
("transposed", features on SBUF partitions) so every projection contracts
over the partition axis. bf16 matmuls with f32 PSUM accumulation.

Per-core pipeline:
  qkvT = WqkvT.T @ xT (q,k transposed-layout; v normal-layout)
  per head: scoresT = kT_h.T @ qT_h; e = exp(s/sqrt(D)) [ACT, bf16]
            causal mask: above-diag blocks fused into exp bias (-30),
            below-diag as per-partition attention_mask scalar, diagonal
            blocks as a full multiplicative {1, e^-30} mask [DVE]
            Z = ones.T @ e [PE]; rz ~ 1/Z [DVE approx]; rz broadcast via
            bf16 ones matmul; ctxT_h = (v_h.T @ e) * rz ; aT += e * rz
  z0T = WoutT.T @ ctxT + bout + xT ; LN1 (stats via 1/1024-ones matmuls)
  hT = relu(W1T.T @ z1T + b1)  (bounced through DRAM)
  ffT = W2T.T @ hT + b2 + z1T ; LN2 -> zT
Outputs zT / aT are written feature-major; the host transposes on gather.
"""

import os
import numpy as np
import ml_dtypes

import concourse.bass as bass
from concourse import bacc
import concourse.tile as tile
import concourse.mybir as mybir
from concourse.bass_utils import run_bass_kernel_spmd

P = 128
E = 1024
H = 8
D = 128
F = 4096
L = 1024
NB = 8          # batch == number of cores
KE = E // P     # 8 e-chunks
KF = F // P     # 32 f-chunks
LT = 512        # l-tile (psum free dim)
NLT = L // LT   # 2
NDIAG = LT // P  # 4 diagonal-band m'-chunks per l-tile
EPS = 1e-5
SCALE = float(1.0 / np.sqrt(D))
MASKV = float(np.exp(np.float32(-30.0)))

f32 = mybir.dt.float32
bf16 = mybir.dt.bfloat16
bfnp = ml_dtypes.bfloat16
AF = mybir.ActivationFunctionType
ALU = mybir.AluOpType


def build():
    nc = bacc.Bacc(None)

    xTf_d = nc.declare_dram_parameter("xTf", [E, L], f32, False)
    xTb_d = nc.declare_dram_parameter("xTb", [E, L], bf16, False)
    maskd_d = nc.declare_dram_parameter("maskd", [12, P, LT], bf16, False)
    wqk_d = nc.declare_dram_parameter("wqk", [16, P, KE, P], bf16, False)
    wv_d = nc.declare_dram_parameter("wv", [NLT, P, KE, LT], bf16, False)
    bqk_d = nc.declare_dram_parameter("bqk", [P, 16], f32, False)
    bv_d = nc.declare_dram_parameter("bv", [E], f32, False)
    wout_d = nc.declare_dram_parameter("wout", [KE, P, KE, P], bf16, False)
    bout_d = nc.declare_dram_parameter("bout", [P, KE], f32, False)
    w1_d = nc.declare_dram_parameter("w1t", [KF, P, KE, P], bf16, False)
    b1_d = nc.declare_dram_parameter("b1t", [P, KF], f32, False)
    w2_d = nc.declare_dram_parameter("w2t", [KF, P, KE, P], bf16, False)
    b2_d = nc.declare_dram_parameter("b2t", [P, KE], f32, False)
    ln_d = nc.declare_dram_parameter("lnp", [P, KE, 4], f32, False)
    z_d = nc.declare_dram_parameter("z_out", [E, L], f32, isOutput=True)
    a_d = nc.declare_dram_parameter("a_out", [L, L], f32, isOutput=True)

    xTf_r = xTf_d[:].rearrange("(ko ki) l -> ki ko l", ki=P)
    xTb_r = xTb_d[:].rearrange("(ko ki) l -> ki ko l", ki=P)
    z_r = z_d[:].rearrange("(ko ki) l -> ki ko l", ki=P)
    a_r = a_d[:].rearrange("(mo mi) l -> mi mo l", mi=P)

    with tile.TileContext(nc) as tc:
        with (
            tc.tile_pool(name="pool", bufs=1) as pool,
            tc.tile_pool(name="psum", bufs=8, space="PSUM") as psum,
            tc.tile_pool(name="dram", bufs=1, space="DRAM") as dram,
        ):
            def big_b(name):
                return pool.tile([P, KE, L], bf16, tag="bigb", bufs=4, name=name)

            def big_f(name):
                return pool.tile([P, KE, L], f32, tag="bigf", bufs=1, name=name)

            def eblk_t(name):
                return pool.tile([P, KE, LT], bf16, tag="eblk", bufs=4, name=name)

            def pw_t(name):
                return pool.tile([P, KE, P], bf16, tag="pw", bufs=3, name=name)

            def pw2_t(name):
                return pool.tile([P, KE, P], bf16, tag="pw2", bufs=4, name=name)

            def pw4_t(name):
                return pool.tile([P, 4, P], bf16, tag="pw4", bufs=6, name=name)

            def pls_t(name):
                return pool.tile([P, LT], f32, tag="pls", bufs=4, name=name)

            def blt_t(name):
                return pool.tile([P, LT], bf16, tag="blt", bufs=3, name=name)

            def hl_t(name):
                return pool.tile([P, LT], bf16, tag="hl", bufs=8, name=name)

            def rzb_t(name):
                return pool.tile([P, LT], bf16, tag="rzb", bufs=3, name=name)

            def ph_t(name):
                return pool.tile([P, L], bf16, tag="ph", bufs=3, name=name)

            def s32_t(name):
                return pool.tile([32, LT], f32, tag="s32", bufs=4, name=name)

            def s32b_t(name):
                return pool.tile([32, LT], bf16, tag="s32b", bufs=4, name=name)

            def ps_t(name):
                return psum.tile([P, LT], f32, tag="ps", bufs=8, name=name)

            # ---------------- constants ----------------
            bqk_sb = pool.tile([P, 16], f32, tag="bqk", name="bqk_sb")
            nc.gpsimd.dma_start(out=bqk_sb, in_=bqk_d[:])
            bout_sb = pool.tile([P, KE], f32, tag="bout", name="bout_sb")
            nc.gpsimd.dma_start(out=bout_sb, in_=bout_d[:])
            b1_sb = pool.tile([P, KF], f32, tag="b1", name="b1_sb")
            nc.gpsimd.dma_start(out=b1_sb, in_=b1_d[:])
            b2_sb = pool.tile([P, KE], f32, tag="b2", name="b2_sb")
            nc.gpsimd.dma_start(out=b2_sb, in_=b2_d[:])
            ln_sb = pool.tile([P, KE, 4], f32, tag="ln", name="ln_sb")
            nc.gpsimd.dma_start(out=ln_sb, in_=ln_d[:])
            bv_bc = pool.tile([P, E], f32, tag="bvbc", name="bv_bc")
            _bv = bv_d[:]
            bv_ap = bass.AP(tensor=_bv.tensor, offset=_bv.offset,
                            ap=[[0, P]] + list(_bv.ap))
            nc.gpsimd.dma_start(out=bv_bc, in_=bv_ap)

            ones_z = pool.tile([P, 32], bf16, tag="onesz", name="ones_z")
            nc.vector.memset(ones_z, 1.0)
            ones_n = pool.tile([P, 32], bf16, tag="onesn", name="ones_n")
            nc.vector.memset(ones_n, 1.0 / 1024.0)
            ones_r = pool.tile([32, P], bf16, tag="onesr", name="ones_r")
            nc.vector.memset(ones_r, 1.0 / 32.0)
            eps_sb = pool.tile([32, 1], f32, tag="eps", name="eps_sb")
            nc.vector.memset(eps_sb, EPS)
            neg30 = pool.tile([P, 1], f32, tag="neg30", name="neg30")
            nc.vector.memset(neg30, -30.0)

            # ---------------- load x ----------------
            xb = big_b("xb")
            nc.sync.dma_start(out=xb, in_=xTb_r)

            # ---------------- QKV: q, k (transposed layout) ----------------
            qT = big_b("qT")
            kT = big_b("kT")
            for oc in range(16):
                wt = pw_t(f"wqk{oc}")
                nc.sync.dma_start(out=wt, in_=wqk_d[oc])
                dst = qT if oc < 8 else kT
                for lt in range(NLT):
                    ls = slice(lt * LT, (lt + 1) * LT)
                    ps = ps_t(f"ps_qk{oc}_{lt}")
                    for kc in range(KE):
                        nc.tensor.matmul(ps, wt[:, kc, :], xb[:, kc, ls],
                                         start=(kc == 0), stop=(kc == KE - 1))
                    nc.scalar.activation(dst[:, oc % 8, ls], ps, AF.Identity,
                                         bias=bqk_sb[:, oc:oc + 1])

            # ---------------- QKV: v (normal layout) ----------------
            vN = big_b("vN")
            for nt in range(NLT):
                ns = slice(nt * LT, (nt + 1) * LT)
                wvt = eblk_t(f"wv{nt}")
                nc.sync.dma_start(out=wvt, in_=wv_d[nt])
                for lc in range(KE):
                    ps = ps_t(f"ps_v{nt}_{lc}")
                    for kc in range(KE):
                        nc.tensor.matmul(ps, xb[:, kc, lc * P:(lc + 1) * P],
                                         wvt[:, kc, :],
                                         start=(kc == 0), stop=(kc == KE - 1))
                    nc.vector.tensor_add(vN[:, lc, ns], ps, bv_bc[:, ns])

            # ---------------- attention ----------------
            # block type per (lt, mc): diag blocks mc in [4lt, 4lt+3] use the
            # full mask; mc > 4lt+3 is fully above the causal diagonal (all
            # e^-30, fused into the exp bias); mc < 4lt is fully below
            # (mask == attention_mask per partition).
            maskd_sb = pool.tile([P, 12, LT], bf16, tag="maskd",
                                 name="maskd_sb")
            for _i in range(12):
                nc.sync.dma_start(out=maskd_sb[:, _i, :], in_=maskd_d[_i])
            ctxT = big_b("ctxT")
            adram = dram.tile([KE, P, L], bf16, name="adram")

            def attn_scores(lt, h):
                ls = slice(lt * LT, (lt + 1) * LT)
                eb = eblk_t(f"e{lt}_{h}")
                for mc in range(KE):
                    ps = ps_t(f"ps_s{lt}{h}{mc}")
                    nc.tensor.matmul(ps, kT[:, h, mc * P:(mc + 1) * P],
                                     qT[:, h, ls], start=True, stop=True)
                    if mc > NDIAG * lt + (NDIAG - 1):       # above diagonal
                        nc.scalar.activation(eb[:, mc, :], ps, AF.Exp,
                                             scale=SCALE, bias=neg30)
                    else:
                        nc.scalar.activation(eb[:, mc, :], ps, AF.Exp,
                                             scale=SCALE)
                        nc.vector.tensor_mul(
                            eb[:, mc, :], eb[:, mc, :],
                            maskd_sb[:, 4 * lt + mc, :])
                return eb

            def attn_post(lt, h, eb):
                ls = slice(lt * LT, (lt + 1) * LT)
                # softmax denominators (column sums of eb)
                psz = ps_t(f"ps_z{lt}{h}")
                for mc in range(KE):
                    nc.tensor.matmul(psz[0:32, :], ones_z, eb[:, mc, :],
                                     start=(mc == 0), stop=(mc == KE - 1))
                rz32 = s32_t(f"rz32_{lt}{h}")
                nc.vector.reciprocal_approx_fast(rz32, psz[0:32, :])
                rz32b = s32b_t(f"rz32b_{lt}{h}")
                nc.vector.tensor_copy(rz32b, rz32)
                psb = ps_t(f"ps_rb{lt}{h}")
                nc.tensor.matmul(psb, ones_r, rz32b, start=True, stop=True)
                rzbc = rzb_t(f"rzbc{lt}{h}")
                nc.scalar.activation(rzbc, psb, AF.Copy)
                # PV on unnormalized weights (independent of the rz chain)
                psc = ps_t(f"ps_c{lt}{h}")
                for mc in range(KE):
                    nc.tensor.matmul(psc, vN[:, mc, h * P:(h + 1) * P],
                                     eb[:, mc, :],
                                     start=(mc == 0), stop=(mc == KE - 1))
                nc.vector.tensor_mul(ctxT[:, h, ls], psc, rzbc)
                # normalize e in place (after PV consumed it); accumulate the
                # attention-weight mean in DRAM via SWDGE read-modify-write
                for mc in range(KE):
                    nc.vector.tensor_mul(eb[:, mc, :], eb[:, mc, :], rzbc)
                    nc.gpsimd.dma_start(
                        out=adram[mc][:, ls], in_=eb[:, mc, :],
                        accum_op=(ALU.bypass if h == 0 else ALU.add))

            # ---------------- layernorm (256-col segment) ----------------
            LSEG = 512
            NSEG = L // LSEG

            def layernorm_seg(src, out_writer, uid, seg):
                """src: [P, KE, L] f32. Column-wise LN over the feature axis
                for one 256-column segment; out_writer(kc, seg, t_ap)."""
                ls = slice(seg * LSEG, (seg + 1) * LSEG)
                s_b = eblk_t(f"snb{uid}{seg}")
                sq_b = eblk_t(f"sqb{uid}{seg}")
                for kc in range(KE):
                    nc.scalar.activation(s_b[:, kc, :LSEG], src[:, kc, ls],
                                         AF.Copy)
                    nc.vector.tensor_mul(sq_b[:, kc, :LSEG], s_b[:, kc, :LSEG],
                                         s_b[:, kc, :LSEG])
                psmu = ps_t(f"ps_mu{uid}{seg}")
                for kc in range(KE):
                    nc.tensor.matmul(psmu[0:32, :LSEG], ones_n,
                                     s_b[:, kc, :LSEG],
                                     start=(kc == 0), stop=(kc == KE - 1))
                psms = ps_t(f"ps_ms{uid}{seg}")
                for kc in range(KE):
                    nc.tensor.matmul(psms[0:32, :LSEG], ones_n,
                                     sq_b[:, kc, :LSEG],
                                     start=(kc == 0), stop=(kc == KE - 1))
                mu32 = s32_t(f"mu32_{uid}{seg}")
                nc.scalar.activation(mu32[:, :LSEG], psmu[0:32, :LSEG], AF.Copy)
                msq = s32_t(f"msq{uid}{seg}")
                nc.vector.tensor_mul(msq[:, :LSEG], mu32[:, :LSEG],
                                     mu32[:, :LSEG])
                var32 = s32_t(f"var32_{uid}{seg}")
                nc.vector.tensor_tensor(var32[:, :LSEG], psms[0:32, :LSEG],
                                        msq[:, :LSEG], ALU.subtract)
                std32 = s32_t(f"std32_{uid}{seg}")
                nc.scalar.activation(std32[:, :LSEG], var32[:, :LSEG], AF.Sqrt,
                                     bias=eps_sb)
                rstd32 = s32_t(f"rstd32_{uid}{seg}")
                nc.vector.reciprocal_approx_fast(rstd32[:, :LSEG],
                                                 std32[:, :LSEG])
                mu32b = s32b_t(f"mu32b_{uid}{seg}")
                nc.vector.tensor_copy(mu32b[:, :LSEG], mu32[:, :LSEG])
                rstd32b = s32b_t(f"rstd32b_{uid}{seg}")
                nc.vector.tensor_copy(rstd32b[:, :LSEG], rstd32[:, :LSEG])
                psb1 = ps_t(f"ps_b1{uid}{seg}")
                nc.tensor.matmul(psb1[:, :LSEG], ones_r, mu32b[:, :LSEG],
                                 start=True, stop=True)
                mu_bc = pls_t(f"mubc{uid}{seg}")
                nc.scalar.activation(mu_bc[:, :LSEG], psb1[:, :LSEG], AF.Copy)
                psb2 = ps_t(f"ps_b2{uid}{seg}")
                nc.tensor.matmul(psb2[:, :LSEG], ones_r, rstd32b[:, :LSEG],
                                 start=True, stop=True)
                rstd_bc = pls_t(f"rstdbc{uid}{seg}")
                nc.scalar.activation(rstd_bc[:, :LSEG], psb2[:, :LSEG], AF.Copy)
                for kc in range(KE):
                    t = pls_t(f"lnt{uid}{seg}{kc}")
                    nc.vector.tensor_tensor(t[:, :LSEG], src[:, kc, ls],
                                            mu_bc[:, :LSEG], ALU.subtract)
                    nc.vector.tensor_mul(t[:, :LSEG], t[:, :LSEG],
                                         rstd_bc[:, :LSEG])
                    out_writer(kc, seg, t[:, :LSEG])

            # ---------------- per-l-tile downstream stages ----------------
            sT = big_f("sT")
            z1b = big_b("z1b")
            s2 = big_f("s2")
            hdram = dram.tile([KF, P, L], bf16, name="hdram")

            def ln1_writer(kc, seg, t):
                ls = slice(seg * LSEG, (seg + 1) * LSEG)
                nc.scalar.activation(z1b[:, kc, ls], t, AF.Identity,
                                     bias=ln_sb[:, kc, 1:2],
                                     scale=ln_sb[:, kc, 0:1])

            def ln2_writer(kc, seg, t):
                ls = slice(seg * LSEG, (seg + 1) * LSEG)
                zc = pls_t(f"zc{kc}{seg}")
                nc.scalar.activation(zc[:, :LSEG], t, AF.Identity,
                                     bias=ln_sb[:, kc, 3:4],
                                     scale=ln_sb[:, kc, 2:3])
                nc.sync.dma_start(out=z_r[:, kc, ls], in_=zc[:, :LSEG])

            def aout_lt(lt):
                ls = slice(lt * LT, (lt + 1) * LT)
                for mc in range(KE):
                    ab = hl_t(f"ab{mc}{lt}")
                    nc.sync.dma_start(out=ab, in_=adram[mc][:, ls])
                    af = pls_t(f"af{mc}{lt}")
                    nc.scalar.activation(af, ab, AF.Copy, scale=1.0 / H)
                    nc.sync.dma_start(out=a_r[:, mc, ls], in_=af)

            def outproj_lt(lt):
                ls = slice(lt * LT, (lt + 1) * LT)
                for oc in range(KE):
                    wt = pw_t(f"wo{lt}{oc}")
                    nc.sync.dma_start(out=wt, in_=wout_d[oc])
                    ps = ps_t(f"ps_o{oc}{lt}")
                    for kc in range(KE):
                        nc.tensor.matmul(ps, wt[:, kc, :], ctxT[:, kc, ls],
                                         start=(kc == 0), stop=(kc == KE - 1))
                    nc.scalar.activation(sT[:, oc, ls], ps, AF.Identity,
                                         bias=bout_sb[:, oc:oc + 1])
                    xc = pls_t(f"xc{oc}{lt}")
                    nc.sync.dma_start(out=xc, in_=xTf_r[:, oc, ls])
                    nc.vector.tensor_add(sT[:, oc, ls], sT[:, oc, ls], xc)

            def mlp1_lt(lt):
                ls = slice(lt * LT, (lt + 1) * LT)
                for fc in range(KF):
                    wt = pw_t(f"w1_{lt}_{fc}")
                    nc.sync.dma_start(out=wt, in_=w1_d[fc])
                    ps = ps_t(f"ps_h{fc}{lt}")
                    for kc in range(KE):
                        nc.tensor.matmul(ps, wt[:, kc, :], z1b[:, kc, ls],
                                         start=(kc == 0), stop=(kc == KE - 1))
                    hc = hl_t(f"hc{fc}{lt}")
                    nc.scalar.activation(hc, ps, AF.Relu,
                                         bias=b1_sb[:, fc:fc + 1])
                    nc.sync.dma_start(out=hdram[fc][:, ls], in_=hc)

            def mlp2_lt(lt):
                ls = slice(lt * LT, (lt + 1) * LT)
                pss = [ps_t(f"ps_ff{lt}{ec}") for ec in range(KE)]
                for kc2 in range(KF):
                    hcl = hl_t(f"hcl{lt}{kc2}")
                    nc.sync.dma_start(out=hcl, in_=hdram[kc2][:, ls])
                    w2t = pw2_t(f"w2_{lt}{kc2}")
                    nc.sync.dma_start(out=w2t, in_=w2_d[kc2])
                    for ec in range(KE):
                        nc.tensor.matmul(pss[ec], w2t[:, ec, :], hcl,
                                         start=(kc2 == 0), stop=(kc2 == KF - 1))
                for ec in range(KE):
                    nc.scalar.activation(s2[:, ec, ls], pss[ec], AF.Identity,
                                         bias=b2_sb[:, ec:ec + 1])
                    nc.vector.tensor_add(s2[:, ec, ls], s2[:, ec, ls],
                                         z1b[:, ec, ls])

            # ---------------- global schedule ----------------
            pending = None
            for lt in range(NLT):
                for h in range(H):
                    eb = attn_scores(lt, h)
                    if pending is not None:
                        attn_post(*pending)
                    pending = (lt, h, eb)
            attn_post(*pending)
            for lt in range(NLT):
                aout_lt(lt)
            for oc in range(KE):
                wt = pw_t(f"wo{oc}")
                nc.sync.dma_start(out=wt, in_=wout_d[oc])
                for lt in range(NLT):
                    ls = slice(lt * LT, (lt + 1) * LT)
                    ps = ps_t(f"ps_o{oc}{lt}")
                    for kc in range(KE):
                        nc.tensor.matmul(ps, wt[:, kc, :], ctxT[:, kc, ls],
                                         start=(kc == 0), stop=(kc == KE - 1))
                    nc.scalar.activation(sT[:, oc, ls], ps, AF.Identity,
                                         bias=bout_sb[:, oc:oc + 1])
                    xc = pls_t(f"xc{oc}{lt}")
                    nc.sync.dma_start(out=xc, in_=xTf_r[:, oc, ls])
                    nc.vector.tensor_add(sT[:, oc, ls], sT[:, oc, ls], xc)
            for seg in range(NSEG):
                layernorm_seg(sT, ln1_writer, "a", seg)
            for fc in range(KF):
                wt = pw_t(f"w1_{fc}")
                nc.sync.dma_start(out=wt, in_=w1_d[fc])
                hc = ph_t(f"hc{fc}")
                for lt in range(NLT):
                    ls = slice(lt * LT, (lt + 1) * LT)
                    ps = ps_t(f"ps_h{fc}{lt}")
                    for kc in range(KE):
                        nc.tensor.matmul(ps, wt[:, kc, :], z1b[:, kc, ls],
                                         start=(kc == 0), stop=(kc == KE - 1))
                    nc.scalar.activation(hc[:, ls], ps, AF.Relu,
                                         bias=b1_sb[:, fc:fc + 1])
                nc.sync.dma_start(out=hdram[fc], in_=hc)
            for lt in range(NLT):
                mlp2_lt(lt)
                layernorm_seg(s2, ln2_writer, "b", lt)

    nc.compile()
    return nc


def _prep_shared(inputs):
    """Host-side weight preparation (shared across cores)."""
    def bfc(a):
        return np.ascontiguousarray(a.astype(bfnp))

    def f32c(a):
        return np.ascontiguousarray(a.astype(np.float32))

    WqkvT = np.asarray(inputs["in_proj_w"], np.float32).T  # (E, 3E)
    wqk = bfc(WqkvT[:, :2 * E].reshape(KE, P, 16, P).transpose(2, 1, 0, 3))
    wv = bfc(WqkvT[:, 2 * E:].reshape(KE, P, NLT, LT).transpose(2, 1, 0, 3))
    ipb = np.asarray(inputs["in_proj_b"], np.float32)
    bqk = f32c(ipb[:2 * E].reshape(16, P).T)
    bv = f32c(ipb[2 * E:])
    WoutT = np.asarray(inputs["out_w"], np.float32).T
    wout = bfc(WoutT.reshape(KE, P, KE, P).transpose(2, 1, 0, 3))
    bout = f32c(np.asarray(inputs["out_b"], np.float32).reshape(KE, P).T)
    W1T = np.asarray(inputs["w1"], np.float32).T  # (E, F)
    w1t = bfc(W1T.reshape(KE, P, KF, P).transpose(2, 1, 0, 3))
    b1t = f32c(np.asarray(inputs["b1"], np.float32).reshape(KF, P).T)
    W2T = np.asarray(inputs["w2"], np.float32).T  # (F, E)
    w2t = bfc(W2T.reshape(KF, P, KE, P))
    b2t = f32c(np.asarray(inputs["b2"], np.float32).reshape(KE, P).T)
    lnp = np.stack([np.asarray(inputs[k], np.float32).reshape(KE, P).T
                    for k in ("ln1_g", "ln1_b", "ln2_g", "ln2_b")], axis=-1)
    lnp = np.ascontiguousarray(lnp)  # [P, KE, 4]
    return dict(wqk=wqk, wv=wv, bqk=bqk, bv=bv, wout=wout, bout=bout,
                w1t=w1t, b1t=b1t, w2t=w2t, b2t=b2t, lnp=lnp)


_TRIL = None


def make_in_map(x_b, am_b, shared):
    """Per-core input dict from one batch element."""
    global _TRIL
    if _TRIL is None:
        _TRIL = np.tril(np.ones((L, L), np.float32))
    xT = np.ascontiguousarray(np.asarray(x_b, np.float32).T)  # (E, L) f32
    keepT = (_TRIL * np.asarray(am_b).astype(np.float32)[None, :]).T  # [m', l]
    maskT = np.where(keepT > 0.5, np.float32(1.0),
                     np.float32(MASKV)).astype(bfnp)
    # diagonal-band blocks only: maskd[lt, :, j, :] is the block for
    # m'-chunk mc = 4*lt + j, columns [lt*LT, (lt+1)*LT)
    maskd = np.empty((12, P, LT), dtype=bfnp)
    for lt in range(NLT):
        for mc in range(NDIAG * (lt + 1)):
            maskd[4 * lt + mc] = maskT[mc * P:(mc + 1) * P,
                                       lt * LT:(lt + 1) * LT]
    m = dict(shared)
    m["xTf"] = xT
    m["xTb"] = np.ascontiguousarray(xT.astype(bfnp))
    m["maskd"] = np.ascontiguousarray(maskd)
    return m


_LAST_RESULT = {}


def kernel(**inputs):
    x = np.asarray(inputs["x"], np.float32)             # (NB, L, E)
    am = np.asarray(inputs["attention_mask"])           # (NB, L)
    shared = _prep_shared(inputs)

    in_maps = [make_in_map(x[b], am[b], shared) for b in range(NB)]

    nc = build()
    trace = os.environ.get("AF_KERNEL_TRACE", "0") == "1"
    res = run_bass_kernel_spmd(nc, in_maps, core_ids=list(range(NB)),
                               trace=trace)
    _LAST_RESULT["exec_time_ns"] = res.exec_time_ns
    _LAST_RESULT["res"] = res

    z = np.stack([np.asarray(res.results[b]["z_out"]).T for b in range(NB)])
    a = np.stack([np.asarray(res.results[b]["a_out"]).T for b in range(NB)])
    return z.astype(np.float32), a.astype(np.float32)
